# revision 5
# baseline (speedup 1.0000x reference)
"""MoE layer (8 experts, top-2 hash routing) on 8 Trainium2 NeuronCores.

Strategy: shard the FFN along the dff axis (4096 -> 8 slices of 512).
Every core computes, for all routed (token, expert) pairs, the partial
FFN contribution of its dff slice:

    z_core[t] = sum_{e in sel(t)} relu(x[t] @ W1[e][:, S] + b1[e][S]) @ W2[e][S, :]

The host sorts tokens by the hash h so each expert's tokens form (at
most) two contiguous runs; per hash-run the two selected experts are
accumulated in PSUM on-device, so each core emits a single [d, T]
partial that the host sums across cores, scales by 1/2, adds the b2
terms to, and un-permutes.

Matmuls run in float32r (full PE rate; values pre-rounded on the host
to the fp32r grid = round-to-nearest-even keeping 11 explicit mantissa
bits). PSUM accumulation is fp32; biases are applied in fp32.

Work/weight traffic is identical on every core: 17.2 GFLOP of matmul
+ ~37 MB weights + 16.8 MB activations in / 16.8 MB out.
"""

import os

import numpy as np

import concourse.bass as bass
import concourse.mybir as mybir
import concourse.tile as tile
from concourse import bacc
from concourse.bass_utils import run_bass_kernel_spmd

# Problem shape (nn_MoELayer: HIDDEN=1024, NUM_EXPERTS=8, TOP_K=2, B=2, S=2048)
D = 1024
DFF = 4096
E = 8
N_CORES = 8
FSL = DFF // N_CORES          # dff slice per core = 512
DC = D // 128                 # 8 contraction chunks for mm1
FC = FSL // 128               # 4 dff chunks per slice
DB = D // 128                 # 8 output-row blocks for mm2
MAX_CHUNK = 512               # token chunk (PSUM bank / fp32 moving limit)

f32 = mybir.dt.float32
f32r = mybir.dt.float32r

LAST_RESULTS = None           # set on each kernel() call (exec stats for test.py)


def _round_fp32r(a: np.ndarray) -> np.ndarray:
    """Round fp32 values to the fp32r grid (RNE, keep 11 explicit mantissa
    bits — matches the hardware's fp32->fp32r cast bit-for-bit)."""
    b = np.ascontiguousarray(a, dtype=np.float32).view(np.uint32).astype(np.uint64)
    keep = b & 0xFFFFF000
    rem = b & 0xFFF
    lsb = (b >> 12) & 1
    up = (rem > 0x800) | ((rem == 0x800) & (lsb == 1))
    out = (keep + (up.astype(np.uint64) << 12)) & 0xFFFFFFFF
    return out.astype(np.uint32).view(np.float32).reshape(a.shape)


def _chunks(start: int, length: int) -> list[tuple[int, int]]:
    """Split [start, start+length) into chunks of <= MAX_CHUNK, keeping every
    chunk >= 256 when length allows (fp32r matmuls hit full rate only for
    moving dims >= 256)."""
    out = []
    pos, rem = start, length
    while rem > 0:
        if rem > MAX_CHUNK + 256 or rem <= MAX_CHUNK:
            c = min(rem, MAX_CHUNK)
        else:  # rem in (MAX_CHUNK, MAX_CHUNK+256]: split so both parts >= 256
            c = rem - 256
        out.append((pos, c))
        pos += c
        rem -= c
    return out


def _try_install_ntff_hook() -> None:
    """Best-effort install of the axon NTFF profile hook (the container's
    antenv package lacks axon_hooks). Only needed when tracing."""
    import sys
    import types

    try:
        import antenv  # noqa: F401

        if "antenv.axon_hooks" in sys.modules:
            return
        mod = types.ModuleType("antenv.axon_hooks")
        _h = {}
        mod.set_axon_ntff_profile_hook = lambda h: _h.__setitem__("h", h)
        mod.get_axon_ntff_profile_hook = lambda: _h.get("h")
        sys.modules["antenv.axon_hooks"] = mod
        antenv.axon_hooks = mod
        from trn_agent_boot.trn_boot import _ntff_profile_via_ctypes

        mod.set_axon_ntff_profile_hook(
            _ntff_profile_via_ctypes("/opt/axon/libaxon_pjrt.so")
        )
        import concourse.bass_utils as bu

        bu.upload_artifacts = lambda tmpdir: f"local:{tmpdir}"
    except Exception:
        pass


def _build_kernel(T: int, run_starts: np.ndarray, run_lens: np.ndarray):
    """Emit the per-core Bass program. All cores run the same program; only
    the weight-slice input data differs."""
    nc = bacc.Bacc("TRN2", target_bir_lowering=False, debug=False,
                   num_devices=N_CORES)

    xt_ext = nc.dram_tensor("xt", [D, T], f32r, kind="ExternalInput")
    w1_ext = nc.dram_tensor("w1", [E, D, FSL], f32r, kind="ExternalInput")
    b1_ext = nc.dram_tensor("b1", [E, FSL], f32, kind="ExternalInput")
    w2_ext = nc.dram_tensor("w2", [E, FSL, D], f32r, kind="ExternalInput")
    zt_ext = nc.dram_tensor("zt", [D, T], f32, kind="ExternalOutput")

    xt_v = xt_ext.ap().rearrange("(c p) t -> p c t", p=128)   # [128, DC, T]
    zt_v = zt_ext.ap().rearrange("(b p) t -> p b t", p=128)   # [128, DB, T]

    with tile.TileContext(nc) as tc:
        with (
            tc.tile_pool(name="wp", bufs=3) as wp,
            tc.tile_pool(name="bp", bufs=1) as bp,
            tc.tile_pool(name="xp", bufs=2) as xp,
            tc.tile_pool(name="hp", bufs=4) as hp,
            tc.tile_pool(name="zp", bufs=2) as zp,
            tc.tile_pool(name="ps1", bufs=4, space="PSUM") as ps1,
            tc.tile_pool(name="ps2", bufs=4, space="PSUM") as ps2,
        ):
            b1_all = bp.tile([128, E, FC], f32, tag="b1")
            nc.sync.dma_start(
                out=b1_all[:],
                in_=b1_ext.ap().rearrange("e (c p) -> p e c", p=128),
            )

            w_tiles: dict[int, tuple] = {}

            def load_expert(e: int):
                w1t = wp.tile([128, DC, FSL], f32r, tag="w1e")
                w2t = wp.tile([128, FC, D], f32r, tag="w2e")
                nc.sync.dma_start(
                    out=w1t[:],
                    in_=w1_ext[e].rearrange("(c p) f -> p c f", p=128),
                )
                nc.sync.dma_start(
                    out=w2t[:],
                    in_=w2_ext[e].rearrange("(c p) d -> p c d", p=128),
                )
                w_tiles[e] = (w1t, w2t)

            load_expert(0)
            load_expert(1)

            for k in range(E):
                if run_lens[k] == 0:
                    continue
                e1, e2 = k, (k + 1) % E
                if e2 not in w_tiles:
                    load_expert(e2)
                w1a, w2a = w_tiles[e1]
                w1b, w2b = w_tiles[e2]
                for (c0, cw) in _chunks(int(run_starts[k]), int(run_lens[k])):
                    xc = xp.tile([128, DC, MAX_CHUNK], f32r, tag="xc")
                    nc.sync.dma_start(out=xc[:, :, :cw], in_=xt_v[:, :, c0:c0 + cw])

                    hts = []
                    for e, w1t in ((e1, w1a), (e2, w1b)):
                        ht = hp.tile([128, FC, MAX_CHUNK], f32r, tag="ht")
                        for fb in range(FC):
                            acc = ps1.tile([128, MAX_CHUNK], f32, tag="acc1")
                            for kd in range(DC):
                                nc.tensor.matmul(
                                    acc[:, :cw],
                                    w1t[:, kd, fb * 128:(fb + 1) * 128],
                                    xc[:, kd, :cw],
                                    start=(kd == 0),
                                    stop=(kd == DC - 1),
                                )
                            nc.scalar.activation(
                                ht[:, fb, :cw],
                                acc[:, :cw],
                                mybir.ActivationFunctionType.Relu,
                                bias=b1_all[:, e, fb:fb + 1],
                            )
                        hts.append(ht)

                    zs = zp.tile([128, DB, MAX_CHUNK], f32, tag="zs")
                    for db in range(DB):
                        acc2 = ps2.tile([128, MAX_CHUNK], f32, tag="acc2")
                        n_acc = 2 * FC
                        i = 0
                        for ht, w2t in ((hts[0], w2a), (hts[1], w2b)):
                            for fb in range(FC):
                                nc.tensor.matmul(
                                    acc2[:, :cw],
                                    w2t[:, fb, db * 128:(db + 1) * 128],
                                    ht[:, fb, :cw],
                                    start=(i == 0),
                                    stop=(i == n_acc - 1),
                                )
                                i += 1
                        nc.vector.tensor_copy(zs[:, db, :cw], acc2[:, :cw])
                    nc.sync.dma_start(out=zt_v[:, :, c0:c0 + cw], in_=zs[:, :, :cw])

                # free the expert that is no longer needed (pool recycles)
                if k >= 1:
                    w_tiles.pop(k, None)

    nc.compile()
    return nc


def kernel(x: np.ndarray, W1: np.ndarray, b1: np.ndarray,
           W2: np.ndarray, b2: np.ndarray) -> np.ndarray:
    global LAST_RESULTS

    x = np.asarray(x, dtype=np.float32)
    W1 = np.asarray(W1, dtype=np.float32)
    b1 = np.asarray(b1, dtype=np.float32)
    W2 = np.asarray(W2, dtype=np.float32)
    b2 = np.asarray(b2, dtype=np.float32)

    B, S, d = x.shape
    assert d == D and W1.shape == (E, D, DFF) and W2.shape == (E, DFF, D)
    T = B * S
    x_flat = x.reshape(T, D)

    # hash routing. Must match the reference's EAGER jnp ops bit-for-bit:
    # on the neuron/axon backend the eager float->int32 astype rounds to
    # nearest (unlike numpy's truncation), so replicate via the same ops.
    try:
        import jax.numpy as jnp

        h = np.asarray(
            jnp.mod(jnp.asarray(x_flat)[:, :2].sum(axis=1).astype(jnp.int32), E)
        ).astype(np.int64)
    except Exception:
        h = np.mod((x_flat[:, 0] + x_flat[:, 1]).astype(np.int32), E).astype(np.int64)

    # sort tokens by h -> contiguous runs per hash value
    perm = np.argsort(h, kind="stable")
    h_sorted = h[perm]
    run_lens = np.bincount(h_sorted, minlength=E)

    # fp32r matmuls require even/aligned free-dim patterns: pad every run to
    # a multiple of 4 tokens with zero columns (their outputs are discarded)
    pad_lens = (-run_lens) % 4
    aug_lens = run_lens + pad_lens
    aug_starts = np.concatenate([[0], np.cumsum(aug_lens)[:-1]])
    T_aug = int(aug_lens.sum())

    x_sorted_T = x_flat[perm].T                               # [D, T]
    xt = np.zeros((D, T_aug), dtype=np.float32)
    col_orig = np.full(T_aug, -1, dtype=np.int64)             # aug col -> sorted idx
    pos = 0
    for k in range(E):
        s, l = pos, int(run_lens[k])
        a0 = int(aug_starts[k])
        xt[:, a0:a0 + l] = x_sorted_T[:, s:s + l]
        col_orig[a0:a0 + l] = np.arange(s, s + l)
        pos += l
    xt = _round_fp32r(xt)

    nc = _build_kernel(T_aug, aug_starts, aug_lens)

    # per-core weight slices along dff
    in_maps = []
    for c in range(N_CORES):
        sl = slice(c * FSL, (c + 1) * FSL)
        in_maps.append({
            "xt": xt,
            "w1": _round_fp32r(np.ascontiguousarray(W1[:, :, sl])),
            "b1": np.ascontiguousarray(b1[:, sl]),
            "w2": _round_fp32r(np.ascontiguousarray(W2[:, sl, :])),
        })

    trace = bool(os.environ.get("MOE_KERNEL_TRACE"))
    if trace:
        _try_install_ntff_hook()
    res = run_bass_kernel_spmd(nc, in_maps, list(range(N_CORES)), trace=trace)
    LAST_RESULTS = res

    # combine: sum partials over cores, drop pad columns, transpose back,
    # halve, add the b2 terms, un-permute
    z = np.zeros((D, T_aug), dtype=np.float32)
    for c in range(N_CORES):
        z += res.results[c]["zt"]
    real = col_orig >= 0
    out_sorted = np.empty((T, D), dtype=np.float32)
    out_sorted[col_orig[real]] = z[:, real].T
    out_sorted *= 0.5
    out_sorted += 0.5 * (b2[h_sorted] + b2[(h_sorted + 1) % E])

    out = np.empty_like(out_sorted)
    out[perm] = out_sorted
    return out.reshape(B, S, D)


# revision 9
# speedup vs baseline: 1.0302x; 1.0302x over previous
"""MoE layer (8 experts, top-2 hash routing) on 8 Trainium2 NeuronCores.

Strategy: shard the FFN along the dff axis (4096 -> 8 slices of 512).
Every core computes, for all routed (token, expert) pairs, the partial
FFN contribution of its dff slice:

    z_core[t] = sum_{e in sel(t)} relu(x[t] @ W1[e][:, S] + b1[e][S]) @ W2[e][S, :]

The host sorts tokens by the hash h so each expert's tokens form (at
most) two contiguous runs; per hash-run the two selected experts are
accumulated in PSUM on-device, so each core emits a single [d, T]
partial that the host sums across cores, scales by 1/2, adds the b2
terms to, and un-permutes.

Matmuls run in float32r (full PE rate; values pre-rounded on the host
to the fp32r grid = round-to-nearest-even keeping 11 explicit mantissa
bits). PSUM accumulation is fp32; biases are applied in fp32.

Work/weight traffic is identical on every core: 17.2 GFLOP of matmul
+ ~37 MB weights + 16.8 MB activations in / 16.8 MB out.
"""

import os

import numpy as np

import concourse.bass as bass
import concourse.mybir as mybir
import concourse.tile as tile
from concourse import bacc
from concourse.bass_utils import run_bass_kernel_spmd

# Problem shape (nn_MoELayer: HIDDEN=1024, NUM_EXPERTS=8, TOP_K=2, B=2, S=2048)
D = 1024
DFF = 4096
E = 8
N_CORES = 8
FSL = DFF // N_CORES          # dff slice per core = 512
DC = D // 128                 # 8 contraction chunks for mm1
FC = FSL // 128               # 4 dff chunks per slice
DB = D // 128                 # 8 output-row blocks for mm2
MAX_CHUNK = 512               # token chunk (PSUM bank / fp32 moving limit)

f32 = mybir.dt.float32
f32r = mybir.dt.float32r

LAST_RESULTS = None           # set on each kernel() call (exec stats for test.py)


def _round_fp32r(a: np.ndarray) -> np.ndarray:
    """Round fp32 values to the fp32r grid (RNE, keep 11 explicit mantissa
    bits — matches the hardware's fp32->fp32r cast bit-for-bit)."""
    b = np.ascontiguousarray(a, dtype=np.float32).view(np.uint32).astype(np.uint64)
    keep = b & 0xFFFFF000
    rem = b & 0xFFF
    lsb = (b >> 12) & 1
    up = (rem > 0x800) | ((rem == 0x800) & (lsb == 1))
    out = (keep + (up.astype(np.uint64) << 12)) & 0xFFFFFFFF
    return out.astype(np.uint32).view(np.float32).reshape(a.shape)


def _chunks(start: int, length: int) -> list[tuple[int, int]]:
    """Split [start, start+length) into chunks of <= MAX_CHUNK, keeping every
    chunk >= 256 when length allows (fp32r matmuls hit full rate only for
    moving dims >= 256)."""
    out = []
    pos, rem = start, length
    while rem > 0:
        if rem > MAX_CHUNK + 256 or rem <= MAX_CHUNK:
            c = min(rem, MAX_CHUNK)
        else:  # rem in (MAX_CHUNK, MAX_CHUNK+256]: split so both parts >= 256
            c = rem - 256
        out.append((pos, c))
        pos += c
        rem -= c
    return out


def _try_install_ntff_hook() -> None:
    """Best-effort install of the axon NTFF profile hook (the container's
    antenv package lacks axon_hooks). Only needed when tracing."""
    import sys
    import types

    try:
        import antenv  # noqa: F401

        if "antenv.axon_hooks" in sys.modules:
            return
        mod = types.ModuleType("antenv.axon_hooks")
        _h = {}
        mod.set_axon_ntff_profile_hook = lambda h: _h.__setitem__("h", h)
        mod.get_axon_ntff_profile_hook = lambda: _h.get("h")
        sys.modules["antenv.axon_hooks"] = mod
        antenv.axon_hooks = mod
        from trn_agent_boot.trn_boot import _ntff_profile_via_ctypes

        mod.set_axon_ntff_profile_hook(
            _ntff_profile_via_ctypes("/opt/axon/libaxon_pjrt.so")
        )
        import concourse.bass_utils as bu

        bu.upload_artifacts = lambda tmpdir: f"local:{tmpdir}"
    except Exception:
        pass


def _build_kernel(T: int, run_starts: np.ndarray, run_lens: np.ndarray):
    """Emit the per-core Bass program. All cores run the same program; only
    the weight-slice input data differs.

    Structure: token sub-chunks (<=512, >=256 where possible) are processed
    in pairs ("super-chunks") so every PE stationary operand is reused for
    both sub-chunks — halving LDWEIGHTS pressure. Weight DMAs are issued as
    contiguous 128-row pieces so the first matmul only waits on ~0.25 MB."""
    nc = bacc.Bacc("TRN2", target_bir_lowering=False, debug=False,
                   num_devices=N_CORES)

    xt_ext = nc.dram_tensor("xt", [D, T], f32r, kind="ExternalInput")
    w1_ext = nc.dram_tensor("w1", [E, D, FSL], f32r, kind="ExternalInput")
    b1_ext = nc.dram_tensor("b1", [E, FSL], f32, kind="ExternalInput")
    w2_ext = nc.dram_tensor("w2", [E, FSL, D], f32r, kind="ExternalInput")
    zt_ext = nc.dram_tensor("zt", [D, T], f32, kind="ExternalOutput")

    xt_v = xt_ext.ap().rearrange("(c p) t -> p c t", p=128)   # [128, DC, T]
    zt_v = zt_ext.ap().rearrange("(b p) t -> p b t", p=128)   # [128, DB, T]

    relu = mybir.ActivationFunctionType.Relu

    with tile.TileContext(nc) as tc:
        with (
            tc.tile_pool(name="wp", bufs=3) as wp,
            tc.tile_pool(name="bp", bufs=1) as bp,
            tc.tile_pool(name="xp", bufs=3) as xp,
            tc.tile_pool(name="hp", bufs=4) as hp,
            tc.tile_pool(name="zp", bufs=2) as zp,
            tc.tile_pool(name="ps1", bufs=4, space="PSUM") as ps1,
            tc.tile_pool(name="ps2", bufs=4, space="PSUM") as ps2,
        ):
            b1_all = bp.tile([128, E, FC], f32, tag="b1")
            nc.sync.dma_start(
                out=b1_all[:],
                in_=b1_ext.ap().rearrange("e (c p) -> p e c", p=128),
            )

            w_tiles: dict[int, tuple] = {}

            def load_expert(e: int):
                w1t = wp.tile([128, DC, FSL], f32r, tag="w1e")
                for kd in range(DC):
                    nc.sync.dma_start(
                        out=w1t[:, kd, :],
                        in_=w1_ext[e, kd * 128:(kd + 1) * 128, :],
                    )
                w2t = wp.tile([128, FC, D], f32r, tag="w2e")
                for fb in range(FC):
                    nc.scalar.dma_start(
                        out=w2t[:, fb, :],
                        in_=w2_ext[e, fb * 128:(fb + 1) * 128, :],
                    )
                w_tiles[e] = (w1t, w2t)

            # super-chunks: pairs of consecutive sub-chunks within each run
            supers = []   # (k, [(c0, cw), ...])
            for k in range(E):
                if run_lens[k] == 0:
                    continue
                subs = _chunks(int(run_starts[k]), int(run_lens[k]))
                for i in range(0, len(subs), 2):
                    supers.append((k, subs[i:i + 2]))

            prev_k = None
            for (k, subs) in supers:
                e1, e2 = k, (k + 1) % E
                if prev_k is not None and k != prev_k:
                    for stale in [e for e in w_tiles if e not in (e1, e2)]:
                        w_tiles.pop(stale)
                for e in (e1, e2):
                    if e not in w_tiles:
                        load_expert(e)
                prev_k = k
                w1a, w2a = w_tiles[e1]
                w1b, w2b = w_tiles[e2]

                xcs = []
                for (c0, cw) in subs:
                    xc = xp.tile([128, DC, MAX_CHUNK], f32r, tag="xc")
                    for kd in range(DC):
                        nc.gpsimd.dma_start(
                            out=xc[:, kd, :cw], in_=xt_v[:, kd, c0:c0 + cw]
                        )
                    xcs.append(xc)

                # mm1 + relu(+b1): ht[e][fb, tok] for both experts
                hts = []
                for e, w1t in ((e1, w1a), (e2, w1b)):
                    ets = []
                    for si, (c0, cw) in enumerate(subs):
                        ht_t = hp.tile([128, FC, MAX_CHUNK], f32r, tag="ht")
                        ets.append(ht_t)
                    for fb in range(FC):
                        accs = [
                            ps1.tile([128, MAX_CHUNK], f32, tag="acc1",
                                     name="acc1")
                            for _ in subs
                        ]
                        for kd in range(DC):
                            for si, (c0, cw) in enumerate(subs):
                                nc.tensor.matmul(
                                    accs[si][:, :cw],
                                    w1t[:, kd, fb * 128:(fb + 1) * 128],
                                    xcs[si][:, kd, :cw],
                                    start=(kd == 0),
                                    stop=(kd == DC - 1),
                                )
                        for si, (c0, cw) in enumerate(subs):
                            nc.scalar.activation(
                                ets[si][:, fb, :cw],
                                accs[si][:, :cw],
                                relu,
                                bias=b1_all[:, e, fb:fb + 1],
                            )
                    hts.append(ets)

                # mm2: zT[d, tok] accumulating both experts over the dff slice
                for half in range(2):
                    zss = [
                        zp.tile([128, DB // 2, MAX_CHUNK], f32, tag="zs",
                                name="zs")
                        for _ in subs
                    ]
                    for dbi in range(DB // 2):
                        db = half * (DB // 2) + dbi
                        accs2 = [
                            ps2.tile([128, MAX_CHUNK], f32, tag="acc2",
                                     name="acc2")
                            for _ in subs
                        ]
                        step = 0
                        for ei, w2t in ((0, w2a), (1, w2b)):
                            for fb in range(FC):
                                for si, (c0, cw) in enumerate(subs):
                                    nc.tensor.matmul(
                                        accs2[si][:, :cw],
                                        w2t[:, fb, db * 128:(db + 1) * 128],
                                        hts[ei][si][:, fb, :cw],
                                        start=(step == 0),
                                        stop=(step == 2 * FC - 1),
                                    )
                                step += 1
                        for si, (c0, cw) in enumerate(subs):
                            nc.vector.tensor_copy(
                                zss[si][:, dbi, :cw], accs2[si][:, :cw]
                            )
                    for si, (c0, cw) in enumerate(subs):
                        nc.scalar.dma_start(
                            out=zt_v[:, half * (DB // 2):(half + 1) * (DB // 2),
                                     c0:c0 + cw],
                            in_=zss[si][:, :, :cw],
                        )

    nc.compile()
    return nc


def kernel(x: np.ndarray, W1: np.ndarray, b1: np.ndarray,
           W2: np.ndarray, b2: np.ndarray) -> np.ndarray:
    global LAST_RESULTS

    x = np.asarray(x, dtype=np.float32)
    W1 = np.asarray(W1, dtype=np.float32)
    b1 = np.asarray(b1, dtype=np.float32)
    W2 = np.asarray(W2, dtype=np.float32)
    b2 = np.asarray(b2, dtype=np.float32)

    B, S, d = x.shape
    assert d == D and W1.shape == (E, D, DFF) and W2.shape == (E, DFF, D)
    T = B * S
    x_flat = x.reshape(T, D)

    # hash routing. Must match the reference's EAGER jnp ops bit-for-bit:
    # on the neuron/axon backend the eager float->int32 astype rounds to
    # nearest (unlike numpy's truncation), so replicate via the same ops.
    try:
        import jax.numpy as jnp

        h = np.asarray(
            jnp.mod(jnp.asarray(x_flat)[:, :2].sum(axis=1).astype(jnp.int32), E)
        ).astype(np.int64)
    except Exception:
        h = np.mod((x_flat[:, 0] + x_flat[:, 1]).astype(np.int32), E).astype(np.int64)

    # sort tokens by h -> contiguous runs per hash value
    perm = np.argsort(h, kind="stable")
    h_sorted = h[perm]
    run_lens = np.bincount(h_sorted, minlength=E)

    # fp32r matmuls require even/aligned free-dim patterns: pad every run to
    # a multiple of 4 tokens with zero columns (their outputs are discarded)
    pad_lens = (-run_lens) % 4
    aug_lens = run_lens + pad_lens
    aug_starts = np.concatenate([[0], np.cumsum(aug_lens)[:-1]])
    T_aug = int(aug_lens.sum())

    x_sorted_T = x_flat[perm].T                               # [D, T]
    xt = np.zeros((D, T_aug), dtype=np.float32)
    col_orig = np.full(T_aug, -1, dtype=np.int64)             # aug col -> sorted idx
    pos = 0
    for k in range(E):
        s, l = pos, int(run_lens[k])
        a0 = int(aug_starts[k])
        xt[:, a0:a0 + l] = x_sorted_T[:, s:s + l]
        col_orig[a0:a0 + l] = np.arange(s, s + l)
        pos += l
    xt = _round_fp32r(xt)

    nc = _build_kernel(T_aug, aug_starts, aug_lens)

    # per-core weight slices along dff
    in_maps = []
    for c in range(N_CORES):
        sl = slice(c * FSL, (c + 1) * FSL)
        in_maps.append({
            "xt": xt,
            "w1": _round_fp32r(np.ascontiguousarray(W1[:, :, sl])),
            "b1": np.ascontiguousarray(b1[:, sl]),
            "w2": _round_fp32r(np.ascontiguousarray(W2[:, sl, :])),
        })

    trace = bool(os.environ.get("MOE_KERNEL_TRACE"))
    if trace:
        _try_install_ntff_hook()
    res = run_bass_kernel_spmd(nc, in_maps, list(range(N_CORES)), trace=trace)
    LAST_RESULTS = res

    # combine: sum partials over cores, drop pad columns, transpose back,
    # halve, add the b2 terms, un-permute
    z = np.zeros((D, T_aug), dtype=np.float32)
    for c in range(N_CORES):
        z += res.results[c]["zt"]
    real = col_orig >= 0
    out_sorted = np.empty((T, D), dtype=np.float32)
    out_sorted[col_orig[real]] = z[:, real].T
    out_sorted *= 0.5
    out_sorted += 0.5 * (b2[h_sorted] + b2[(h_sorted + 1) % E])

    out = np.empty_like(out_sorted)
    out[perm] = out_sorted
    return out.reshape(B, S, D)
